# revision 7
# baseline (speedup 1.0000x reference)
"""MeshGraphEncoder (g2m) Trainium2 kernel — 8-core SPMD, edge-parallel by dst.

Strategy
--------
Host: sort edges by dst_idx. Core c owns mesh rows [2500c, 2500(c+1)) and the
edges pointing into them, plus grid rows [10000c, 10000(c+1)). Node features /
weights needed by a core's edges are gathered & concatenated on host into the
edge-MLP input (edge-parallel sharding materialization), pre-transposed and
cast to bf16 so the device runs a pure dense pipeline.

Device (per core, all bf16 matmuls with f32 PSUM accumulation):
  edge MLP:  h[hid, e] = silu(W1^T @ e_in)   (weights stationary, e streams)
             y[e, d]   = (h-slices)^T @ W2   (edges on partitions for LN)
             efeat     = LayerNorm(y) along free dim (bn_stats/bn_aggr)
  seg-sum:   one-hot matmul per 128-edge tile accumulated into a PSUM window
             of <=128 mesh rows; window placed into a DRAM agg buffer via
             indirect scatter-DMA (indices are data => identical instruction
             stream across cores despite data-dependent row placement).
  dst MLP:   cat[agg | mesh] -> PE-transpose -> MLP -> LN -> +residual
  src MLP:   grid -> MLP -> LN -> +residual (input pre-transposed on host)

Edges per core are padded to 21 windows x 9 tiles x 128 = 24192 slots; pad
edges have zero e_in and all-zero one-hot columns so they contribute nothing.
"""

from contextlib import ExitStack

import numpy as np
import ml_dtypes

# ---------------------------------------------------------------- constants
D = 256          # feature dim
HID = 256        # MLP hidden dim
NG = 80000       # grid nodes
NM = 20000       # mesh nodes
E = 160000       # edges
EPS = 1e-5
NCORES = 8

NM_C = NM // NCORES          # 2500 mesh rows / core
NG_C = NG // NCORES          # 10000 grid rows / core
W = 21                       # psum windows / core
K_TILES = 9                  # 128-edge tiles / window
CAP = K_TILES * 128          # 1152 edge slots / window
T_EDGE = W * K_TILES         # 189 edge tiles / core
E_C = T_EDGE * 128           # 24192 edge slots / core
CHUNK_T = 3                  # edge tiles per L1 chunk
CHUNK_E = CHUNK_T * 128      # 384 edges / chunk
NCH = T_EDGE // CHUNK_T      # 63 chunks / core
GT = (NG_C + 127) // 128     # 79 grid tiles (last has 16 real rows)
NG_PAD = GT * 128            # 10112
MT = (NM_C + 127) // 128     # 20 mesh tiles (last has 68 real rows)
NM_PAD = MT * 128            # 2560
TRASH = NM_C                 # agg scratch trash row

BF16 = ml_dtypes.bfloat16

_MODULE_CACHE = {}


# ---------------------------------------------------------------- device IR
def _build_module(flags):
    """flags: (eb1,eb2,egb, sb1,sb2,sgb, db1,db2,dgb) booleans — whether the
    corresponding bias / gamma-beta paths are non-trivial."""
    import concourse.bass as bass
    import concourse.tile as tile
    from concourse import bacc, mybir
    from concourse.masks import make_identity

    f32 = mybir.dt.float32
    bf16 = mybir.dt.bfloat16
    i32 = mybir.dt.int32
    AF = mybir.ActivationFunctionType
    ALU = mybir.AluOpType

    eb1f, eb2f, egbf, sb1f, sb2f, sgbf, db1f, db2f, dgbf = flags

    nc = bacc.Bacc("TRN2", target_bir_lowering=False, debug=False,
                   num_devices=NCORES)

    # ---------------- DRAM tensors
    einT = nc.dram_tensor("einT", [NCH, 128, 6, CHUNK_E], bf16, kind="ExternalInput")
    oneh = nc.dram_tensor("oneh", [T_EDGE, 128, 128], bf16, kind="ExternalInput")
    scat = nc.dram_tensor("scat", [W, 128, 1], i32, kind="ExternalInput")
    eW1 = nc.dram_tensor("eW1t", [128, 6, 2, 128], bf16, kind="ExternalInput")
    eW2 = nc.dram_tensor("eW2t", [128, 2, D], bf16, kind="ExternalInput")
    dW1 = nc.dram_tensor("dW1t", [128, 4, HID], bf16, kind="ExternalInput")
    dW2 = nc.dram_tensor("dW2t", [128, 2, D], bf16, kind="ExternalInput")
    sW1 = nc.dram_tensor("sW1t", [128, 2, HID], bf16, kind="ExternalInput")
    sW2 = nc.dram_tensor("sW2t", [128, 2, D], bf16, kind="ExternalInput")
    gridT = nc.dram_tensor("gridT", [GT, 128, 2, 128], bf16, kind="ExternalInput")
    grid32 = nc.dram_tensor("grid32", [NG_PAD, D], f32, kind="ExternalInput")
    mesh32 = nc.dram_tensor("mesh32", [NM_PAD, D], f32, kind="ExternalInput")

    # optional bias / gamma-beta inputs (replicated across partitions on host
    # where they live on the free axis)
    opt = {}
    if eb1f:
        opt["eb1t"] = nc.dram_tensor("eb1t", [128, 2], f32, kind="ExternalInput")
    if eb2f:
        opt["eb2r"] = nc.dram_tensor("eb2r", [128, D], f32, kind="ExternalInput")
    if egbf:
        opt["egr"] = nc.dram_tensor("egr", [128, D], f32, kind="ExternalInput")
        opt["ebetar"] = nc.dram_tensor("ebetar", [128, D], f32, kind="ExternalInput")
    if sb1f:
        opt["sb1r"] = nc.dram_tensor("sb1r", [128, HID], f32, kind="ExternalInput")
    if sb2f:
        opt["sb2r"] = nc.dram_tensor("sb2r", [128, D], f32, kind="ExternalInput")
    if sgbf:
        opt["sgr"] = nc.dram_tensor("sgr", [128, D], f32, kind="ExternalInput")
        opt["sbetar"] = nc.dram_tensor("sbetar", [128, D], f32, kind="ExternalInput")
    if db1f:
        opt["db1r"] = nc.dram_tensor("db1r", [128, HID], f32, kind="ExternalInput")
    if db2f:
        opt["db2r"] = nc.dram_tensor("db2r", [128, D], f32, kind="ExternalInput")
    if dgbf:
        opt["dgr"] = nc.dram_tensor("dgr", [128, D], f32, kind="ExternalInput")
        opt["dbetar"] = nc.dram_tensor("dbetar", [128, D], f32, kind="ExternalInput")

    grid_out = nc.dram_tensor("grid_out", [NG_C, D], f32, kind="ExternalOutput")
    mesh_out = nc.dram_tensor("mesh_out", [NM_C, D], f32, kind="ExternalOutput")

    with tile.TileContext(nc) as tc, ExitStack() as ctx:
        const = ctx.enter_context(tc.tile_pool(name="const", bufs=1))
        ein_p = ctx.enter_context(tc.tile_pool(name="ein", bufs=3))
        oneh_p = ctx.enter_context(tc.tile_pool(name="oneh", bufs=4))
        hsb_p = ctx.enter_context(tc.tile_pool(name="hsb", bufs=3))
        ef_p = ctx.enter_context(tc.tile_pool(name="ef", bufs=3))
        ln_p = ctx.enter_context(tc.tile_pool(name="ln", bufs=4))
        io_p = ctx.enter_context(tc.tile_pool(name="io", bufs=3))
        ps_h = ctx.enter_context(tc.tile_pool(name="ps_h", bufs=2, space="PSUM"))
        ps_y = ctx.enter_context(tc.tile_pool(name="ps_y", bufs=2, space="PSUM"))
        ps_w = ctx.enter_context(tc.tile_pool(name="ps_w", bufs=2, space="PSUM"))
        dram = ctx.enter_context(tc.tile_pool(name="dram", bufs=1, space="DRAM"))

        agg = dram.tile([NM_PAD, D], f32)

        # ----- constants
        eW1_sb = const.tile([128, 6, 2, 128], bf16)
        nc.sync.dma_start(eW1_sb[:], eW1[:])
        eW2_sb = const.tile([128, 2, D], bf16)
        nc.sync.dma_start(eW2_sb[:], eW2[:])
        dW1_sb = const.tile([128, 4, HID], bf16)
        nc.sync.dma_start(dW1_sb[:], dW1[:])
        dW2_sb = const.tile([128, 2, D], bf16)
        nc.sync.dma_start(dW2_sb[:], dW2[:])
        sW1_sb = const.tile([128, 2, HID], bf16)
        nc.sync.dma_start(sW1_sb[:], sW1[:])
        sW2_sb = const.tile([128, 2, D], bf16)
        nc.sync.dma_start(sW2_sb[:], sW2[:])
        eps_sb = const.tile([128, 1], f32)
        nc.vector.memset(eps_sb[:], EPS)
        ident = const.tile([128, 128], bf16)
        make_identity(nc, ident[:])
        opt_sb = {}
        for name, t in opt.items():
            s = const.tile(list(t.shape), f32)
            nc.sync.dma_start(s[:], t[:])
            opt_sb[name] = s

        def layernorm_to(out_ap, y_ps, gbf, gname, bname):
            """LN along free dim of y_ps [128, D] -> out_ap (any dtype)."""
            stats = ln_p.tile([128, 6], f32, tag="stats")
            nc.vector.bn_stats(stats[:], y_ps[:])
            mv = ln_p.tile([128, 2], f32, tag="mv")
            nc.vector.bn_aggr(mv[:], stats[:])
            std = ln_p.tile([128, 1], f32, tag="std")
            nc.scalar.activation(std[:], mv[:, 1:2], AF.Sqrt, bias=eps_sb[:, :])
            rstd = ln_p.tile([128, 1], f32, tag="rstd")
            nc.vector.reciprocal(rstd[:], std[:])
            if gbf:
                t = ln_p.tile([128, D], f32, tag="lnt")
                nc.vector.tensor_scalar(
                    out=t[:], in0=y_ps[:], scalar1=mv[:, 0:1], scalar2=rstd[:],
                    op0=ALU.subtract, op1=ALU.mult)
                nc.vector.tensor_mul(t[:], t[:], opt_sb[gname][:])
                nc.vector.tensor_add(out_ap, t[:], opt_sb[bname][:])
            else:
                nc.vector.tensor_scalar(
                    out=out_ap, in0=y_ps[:], scalar1=mv[:, 0:1], scalar2=rstd[:],
                    op0=ALU.subtract, op1=ALU.mult)

        # zero the agg tail (rows >= NM_C that no window scatters into)
        zt = io_p.tile([128, D], f32, tag="wtmp")
        nc.vector.memset(zt[:], 0.0)
        nc.sync.dma_start(agg[NM_PAD - 128:NM_PAD, :], zt[:])

        # ================= edge phase =================
        for w in range(W):
            win_ps = ps_w.tile([128, D], f32, tag="win")
            for c3 in range(CHUNK_T):
                ch = w * CHUNK_T + c3
                ein_sb = ein_p.tile([128, 6, CHUNK_E], bf16, tag="ein")
                nc.sync.dma_start(ein_sb[:], einT[ch])
                h_sbs = []
                for j2 in range(2):
                    h_ps = ps_h.tile([128, CHUNK_E], f32, tag="h_ps")
                    for k in range(6):
                        nc.tensor.matmul(
                            h_ps[:], lhsT=eW1_sb[:, k, j2, :], rhs=ein_sb[:, k, :],
                            start=(k == 0), stop=(k == 5))
                    h_sb = hsb_p.tile([128, CHUNK_E], bf16, tag="h_sb")
                    if eb1f:
                        nc.scalar.activation(h_sb[:], h_ps[:], AF.Silu,
                                             bias=opt_sb["eb1t"][:, j2:j2 + 1])
                    else:
                        nc.scalar.activation(h_sb[:], h_ps[:], AF.Silu)
                    h_sbs.append(h_sb)
                for s in range(CHUNK_T):
                    ki = c3 * CHUNK_T + s           # tile index in window
                    t_idx = ch * CHUNK_T + s        # global edge-tile index
                    y_ps = ps_y.tile([128, D], f32, tag="y_ps")
                    for j2 in range(2):
                        nc.tensor.matmul(
                            y_ps[:], lhsT=h_sbs[j2][:, s * 128:(s + 1) * 128],
                            rhs=eW2_sb[:, j2, :],
                            start=(j2 == 0), stop=(j2 == 1))
                    if eb2f:
                        nc.vector.tensor_add(y_ps[:], y_ps[:], opt_sb["eb2r"][:])
                    ef_sb = ef_p.tile([128, D], bf16, tag="ef")
                    layernorm_to(ef_sb[:], y_ps, egbf, "egr", "ebetar")
                    oh_sb = oneh_p.tile([128, 128], bf16, tag="oh")
                    nc.sync.dma_start(oh_sb[:], oneh[t_idx])
                    nc.tensor.matmul(
                        win_ps[:], lhsT=oh_sb[:], rhs=ef_sb[:],
                        start=(ki == 0), stop=(ki == K_TILES - 1))
            # close window: copy out of PSUM, scatter rows to agg by index
            tmp = io_p.tile([128, D], f32, tag="wtmp")
            nc.scalar.copy(tmp[:], win_ps[:])
            idx_sb = ln_p.tile([128, 1], i32, tag="scatidx")
            nc.sync.dma_start(idx_sb[:], scat[w])
            nc.gpsimd.indirect_dma_start(
                out=agg[:, :],
                out_offset=bass.IndirectOffsetOnAxis(ap=idx_sb[:, :1], axis=0),
                in_=tmp[:], in_offset=None)

        # ================= mesh (dst) MLP =================
        for m in range(MT):
            rows = min(128, NM_C - m * 128)
            agg_sb = io_p.tile([128, D], f32, tag="agg")
            nc.sync.dma_start(agg_sb[:], agg[m * 128:(m + 1) * 128, :])
            mesh_sb = io_p.tile([128, D], f32, tag="mesh")
            nc.sync.dma_start(mesh_sb[:], mesh32[m * 128:(m + 1) * 128, :])
            cat_sb = io_p.tile([128, 2 * D], bf16, tag="cat")
            nc.vector.tensor_copy(cat_sb[:, 0:D], agg_sb[:])
            nc.vector.tensor_copy(cat_sb[:, D:2 * D], mesh_sb[:])
            h_ps = ps_h.tile([128, HID], f32, tag="h_ps")
            for k4 in range(4):
                tr_ps = ps_w.tile([128, 128], bf16, tag="tr")
                nc.tensor.transpose(
                    tr_ps[:], cat_sb[:, k4 * 128:(k4 + 1) * 128], ident[:])
                catT_sb = hsb_p.tile([128, 128], bf16, tag="catT")
                nc.scalar.copy(catT_sb[:], tr_ps[:])
                nc.tensor.matmul(
                    h_ps[:], lhsT=catT_sb[:], rhs=dW1_sb[:, k4, :],
                    start=(k4 == 0), stop=(k4 == 3))
            if db1f:
                nc.vector.tensor_add(h_ps[:], h_ps[:], opt_sb["db1r"][:])
            h_sb = hsb_p.tile([128, HID], bf16, tag="mlp_h")
            nc.scalar.activation(h_sb[:], h_ps[:], AF.Silu)
            y_ps = ps_y.tile([128, D], f32, tag="y_ps")
            for j2 in range(2):
                tr_ps = ps_w.tile([128, 128], bf16, tag="tr")
                nc.tensor.transpose(
                    tr_ps[:], h_sb[:, j2 * 128:(j2 + 1) * 128], ident[:])
                hT_sb = hsb_p.tile([128, 128], bf16, tag="catT")
                nc.scalar.copy(hT_sb[:], tr_ps[:])
                nc.tensor.matmul(
                    y_ps[:], lhsT=hT_sb[:], rhs=dW2_sb[:, j2, :],
                    start=(j2 == 0), stop=(j2 == 1))
            if db2f:
                nc.vector.tensor_add(y_ps[:], y_ps[:], opt_sb["db2r"][:])
            ln_sb = ef_p.tile([128, D], f32, tag="lnout")
            layernorm_to(ln_sb[:], y_ps, dgbf, "dgr", "dbetar")
            out_sb = io_p.tile([128, D], f32, tag="mout")
            nc.vector.tensor_add(out_sb[:], ln_sb[:], mesh_sb[:])
            nc.sync.dma_start(mesh_out[m * 128:m * 128 + rows, :], out_sb[:rows, :])

        # ================= grid (src) MLP =================
        for t in range(GT):
            rows = min(128, NG_C - t * 128)
            gT_sb = io_p.tile([128, 2, 128], bf16, tag="gT")
            nc.sync.dma_start(gT_sb[:], gridT[t])
            g32_sb = io_p.tile([128, D], f32, tag="g32")
            nc.sync.dma_start(g32_sb[:], grid32[t * 128:(t + 1) * 128, :])
            h_ps = ps_h.tile([128, HID], f32, tag="h_ps")
            for k2 in range(2):
                nc.tensor.matmul(
                    h_ps[:], lhsT=gT_sb[:, k2, :], rhs=sW1_sb[:, k2, :],
                    start=(k2 == 0), stop=(k2 == 1))
            if sb1f:
                nc.vector.tensor_add(h_ps[:], h_ps[:], opt_sb["sb1r"][:])
            h_sb = hsb_p.tile([128, HID], bf16, tag="mlp_h")
            nc.scalar.activation(h_sb[:], h_ps[:], AF.Silu)
            y_ps = ps_y.tile([128, D], f32, tag="y_ps")
            for j2 in range(2):
                tr_ps = ps_w.tile([128, 128], bf16, tag="tr")
                nc.tensor.transpose(
                    tr_ps[:], h_sb[:, j2 * 128:(j2 + 1) * 128], ident[:])
                hT_sb = hsb_p.tile([128, 128], bf16, tag="catT")
                nc.scalar.copy(hT_sb[:], tr_ps[:])
                nc.tensor.matmul(
                    y_ps[:], lhsT=hT_sb[:], rhs=sW2_sb[:, j2, :],
                    start=(j2 == 0), stop=(j2 == 1))
            if sb2f:
                nc.vector.tensor_add(y_ps[:], y_ps[:], opt_sb["sb2r"][:])
            ln_sb = ef_p.tile([128, D], f32, tag="lnout")
            layernorm_to(ln_sb[:], y_ps, sgbf, "sgr", "sbetar")
            out_sb = io_p.tile([128, D], f32, tag="gout")
            nc.vector.tensor_add(out_sb[:], ln_sb[:], g32_sb[:])
            nc.sync.dma_start(grid_out[t * 128:t * 128 + rows, :], out_sb[:rows, :])

    nc.finalize()
    return nc


def get_module(flags):
    key = tuple(flags)
    if key not in _MODULE_CACHE:
        _MODULE_CACHE[key] = _build_module(key)
    return _MODULE_CACHE[key]


# ---------------------------------------------------------------- host prep
def _edge_layout(dst_idx):
    """Sort edges by dst, split per core, cut PSUM windows.

    Returns per-core dicts with:
      eids  [E_C]  int64 global edge id per slot (-1 = pad)
      ldst  [E_C]  local dst row per slot (-1 = pad)
      wrows [W, 2] (row_start, row_end) per window
    """
    order = np.argsort(dst_idx, kind="stable")
    dsts = dst_idx[order]
    bounds = np.searchsorted(dsts, np.arange(NCORES + 1) * NM_C)
    cores = []
    for c in range(NCORES):
        lo, hi = bounds[c], bounds[c + 1]
        eids_c = order[lo:hi]
        ldst_c = dsts[lo:hi].astype(np.int64) - c * NM_C
        counts = np.bincount(ldst_c, minlength=NM_C)
        assert counts.max() <= CAP, "single mesh row exceeds window capacity"
        # greedy windows: <=128 rows and <=CAP edges each
        wrows = []
        wedges = []
        r = 0
        epos = 0
        while r < NM_C:
            cum = np.cumsum(counts[r:r + 128])
            nrows = int(np.searchsorted(cum, CAP, side="right"))
            nrows = max(1, min(nrows, 128))
            ne = int(cum[nrows - 1])
            wrows.append((r, r + nrows))
            wedges.append((epos, epos + ne))
            r += nrows
            epos += ne
        assert len(wrows) <= W, f"needs {len(wrows)} windows > {W}"
        while len(wrows) < W:
            wrows.append((NM_C, NM_C))
            wedges.append((epos, epos))
        # slot arrays
        eids = np.full(E_C, -1, np.int64)
        ldst = np.full(E_C, -1, np.int64)
        for w in range(W):
            e0, e1 = wedges[w]
            n = e1 - e0
            assert n <= CAP
            base = w * CAP
            eids[base:base + n] = eids_c[e0:e1]
            ldst[base:base + n] = ldst_c[e0:e1]
        cores.append(dict(eids=eids, ldst=ldst,
                          wrows=np.array(wrows, np.int64)))
    return cores


def build_in_maps(inputs):
    g2m_efeat = np.asarray(inputs["g2m_efeat"], np.float32)
    grid_nfeat = np.asarray(inputs["grid_nfeat"], np.float32)
    mesh_nfeat = np.asarray(inputs["mesh_nfeat"], np.float32)
    src_idx = np.asarray(inputs["src_idx"])
    dst_idx = np.asarray(inputs["dst_idx"])

    def rep(v):  # replicate a free-axis vector across 128 partitions
        return np.ascontiguousarray(
            np.broadcast_to(np.asarray(v, np.float32), (128, v.shape[0])))

    eb1 = np.asarray(inputs["eb1"], np.float32)
    flags = (
        bool(np.any(inputs["eb1"])), bool(np.any(inputs["eb2"])),
        bool(np.any(np.asarray(inputs["eg"]) != 1) or np.any(inputs["ebeta"])),
        bool(np.any(inputs["sb1"])), bool(np.any(inputs["sb2"])),
        bool(np.any(np.asarray(inputs["sg"]) != 1) or np.any(inputs["sbeta"])),
        bool(np.any(inputs["db1"])), bool(np.any(inputs["db2"])),
        bool(np.any(np.asarray(inputs["dg"]) != 1) or np.any(inputs["dbeta"])),
    )

    # weights, pre-tiled (shared by all cores)
    def tile_w(wm, kparts):
        k, n = wm.shape
        assert k == kparts * 128
        return np.ascontiguousarray(
            wm.reshape(kparts, 128, n).transpose(1, 0, 2).astype(BF16))

    eW1t = np.ascontiguousarray(
        np.asarray(inputs["eW1"], np.float32)
        .reshape(6, 128, 2, 128).transpose(1, 0, 2, 3).astype(BF16))
    eW2t = tile_w(np.asarray(inputs["eW2"], np.float32), 2)
    dW1t = tile_w(np.asarray(inputs["dW1"], np.float32), 4)
    dW2t = tile_w(np.asarray(inputs["dW2"], np.float32), 2)
    sW1t = tile_w(np.asarray(inputs["sW1"], np.float32), 2)
    sW2t = tile_w(np.asarray(inputs["sW2"], np.float32), 2)

    shared = dict(eW1t=eW1t, eW2t=eW2t, dW1t=dW1t, dW2t=dW2t,
                  sW1t=sW1t, sW2t=sW2t)
    eb1f, eb2f, egbf, sb1f, sb2f, sgbf, db1f, db2f, dgbf = flags
    if eb1f:
        shared["eb1t"] = np.ascontiguousarray(
            np.asarray(inputs["eb1"], np.float32).reshape(2, 128).T)
    if eb2f:
        shared["eb2r"] = rep(np.asarray(inputs["eb2"], np.float32))
    if egbf:
        shared["egr"] = rep(np.asarray(inputs["eg"], np.float32))
        shared["ebetar"] = rep(np.asarray(inputs["ebeta"], np.float32))
    if sb1f:
        shared["sb1r"] = rep(np.asarray(inputs["sb1"], np.float32))
    if sb2f:
        shared["sb2r"] = rep(np.asarray(inputs["sb2"], np.float32))
    if sgbf:
        shared["sgr"] = rep(np.asarray(inputs["sg"], np.float32))
        shared["sbetar"] = rep(np.asarray(inputs["sbeta"], np.float32))
    if db1f:
        shared["db1r"] = rep(np.asarray(inputs["db1"], np.float32))
    if db2f:
        shared["db2r"] = rep(np.asarray(inputs["db2"], np.float32))
    if dgbf:
        shared["dgr"] = rep(np.asarray(inputs["dg"], np.float32))
        shared["dbetar"] = rep(np.asarray(inputs["dbeta"], np.float32))

    cores = _edge_layout(dst_idx)
    in_maps = []
    for c in range(NCORES):
        cd = cores[c]
        eids, ldst, wrows = cd["eids"], cd["ldst"], cd["wrows"]
        valid = eids >= 0
        ve = eids[valid]

        # e_in = [efeat | grid[src] | mesh[dst]]  (pads = 0)
        ein = np.zeros((E_C, 3 * D), np.float32)
        ein[valid, 0:D] = g2m_efeat[ve]
        ein[valid, D:2 * D] = grid_nfeat[src_idx[ve]]
        ein[valid, 2 * D:3 * D] = mesh_nfeat[dst_idx[ve]]
        einT = np.ascontiguousarray(
            ein.reshape(NCH, CHUNK_E, 6, 128).transpose(0, 3, 2, 1).astype(BF16))
        del ein

        # one-hot seg matrices + scatter indices
        oneh = np.zeros((T_EDGE, 128, 128), BF16)
        scat = np.full((W, 128, 1), TRASH, np.int32)
        iot = np.arange(128)
        for w in range(W):
            r0, r1 = wrows[w]
            nr = r1 - r0
            scat[w, :nr, 0] = r0 + iot[:nr]
            for kt in range(K_TILES):
                t_idx = w * K_TILES + kt
                sl = slice(w * CAP + kt * 128, w * CAP + (kt + 1) * 128)
                ld = ldst[sl]
                ok = ld >= 0
                oneh[t_idx][ok] = (
                    (ld[ok, None] - r0) == iot[None, :]).astype(BF16)

        # grid slices
        gsl = grid_nfeat[c * NG_C:(c + 1) * NG_C]
        g32 = np.zeros((NG_PAD, D), np.float32)
        g32[:NG_C] = gsl
        gT = np.ascontiguousarray(
            g32.reshape(GT, 128, 2, 128).transpose(0, 3, 2, 1).astype(BF16))

        m32 = np.zeros((NM_PAD, D), np.float32)
        m32[:NM_C] = mesh_nfeat[c * NM_C:(c + 1) * NM_C]

        im = dict(shared)
        im.update(einT=einT, oneh=oneh, scat=scat, gridT=gT,
                  grid32=g32, mesh32=m32)
        in_maps.append(im)
    return in_maps, flags


def run_spmd(inputs, **kw):
    from concourse.bass_utils import run_bass_kernel_spmd

    in_maps, flags = build_in_maps(inputs)
    nc = get_module(flags)
    res = run_bass_kernel_spmd(nc, in_maps, core_ids=list(range(NCORES)), **kw)
    grid_out = np.concatenate([res.results[c]["grid_out"] for c in range(NCORES)])
    mesh_out = np.concatenate([res.results[c]["mesh_out"] for c in range(NCORES)])
    return (grid_out, mesh_out), res


def kernel(**inputs):
    out, _ = run_spmd(inputs)
    return out
